# revision 30
# baseline (speedup 1.0000x reference)
"""Trainium2 Bass kernel for nn_Cross_Attention (dual cross channel-attention block).

Architecture (8 NeuronCores, data-parallel):
  core i -> (batch b = i//2, row-half h = i%2) of the 4x[64,256,256] images.

Math restructuring (exact, up to float assoc):
  qkv = dwconv3x3(conv1x1(x, W))  is computed with the 3x3 depthwise conv
  *folded* into the 1x1 conv: 9 PSUM-accumulated matmuls whose moving operand
  is the (zero-padded) input shifted by the tap offset.
  Channel attention needs only second moments of q,k:
     S_a[c,d] = sum_p qb[c,p] ka[d,p],  S_b[c,d] = sum_p qa[c,p] kb[d,p]
     n_*[c]   = sum_p q[c,p]^2
  computed on-chip (Gram via DMA-transposed bf16 operands + PE matmuls,
  norms via ScalarE Square+accum), then AllReduce'd across the 2 cores
  sharing a batch. Softmax + all downstream linear layers are folded into
  10 per-batch [128,64] stationaries applied in one output pass:
     out = sum_t S2A_t^T @ x_shift_t + S2B_t^T @ y_shift_t + CA^T@x + CB^T@y
  where S2A_t[xc,o] = sum_d WvA[d,xc] * (W1 @ blockdiag(attn_a))[o,d] * dwvA[d,t]
  and W1 = concat_w[:, :64] @ proj_A_w  (host-precomputed), etc.
"""

import os
import sys

sys.path.insert(0, "/opt/trn_rl_repo")

import ml_dtypes
import numpy as np

BF16_NP = ml_dtypes.bfloat16

import concourse.bass as bass
import concourse.bacc as bacc
import concourse.tile as tile
from concourse import mybir
from concourse.bass_utils import run_bass_kernel_spmd
from concourse.masks import make_identity

F32 = mybir.dt.float32
F32R = mybir.dt.float32r
BF16 = mybir.dt.bfloat16

B, C, H, W = 4, 64, 256, 256
HEADS, CH = 8, 8
WP = W + 2          # zero-padded width
N_CORES = 8
R_LOC = H // 2      # output rows per core
BLK = 8             # rows per streaming block
TAPS = [(dy, dx) for dy in (-1, 0, 1) for dx in (-1, 0, 1)]
GROUPS = [[0, 1], [2, 3], [4, 5], [6, 7]]


def kernel_body(tc, outs, ins, cfg):
    nc = tc.nc
    rows = cfg["rows"]
    blk = cfg["blk"]
    nblk = rows // blk
    w = cfg["w"]
    wp = w + 2
    groups = cfg["groups"]
    nch_blk = blk * w // 128  # 128-px transpose chunks per block

    xy = ins["xy"]            # [128, rows+2, wp] dram (x on 0:64, y on 64:128)
    out_d = outs["out"]       # [64, rows, w] dram

    from contextlib import ExitStack

    with ExitStack() as ctx:
        consts = ctx.enter_context(tc.tile_pool(name="consts", bufs=1))
        xin = ctx.enter_context(tc.tile_pool(name="xin", bufs=2))
        qkev = ctx.enter_context(tc.tile_pool(name="qkev", bufs=2))
        qkt = ctx.enter_context(tc.tile_pool(name="qkt", bufs=2))
        obuf = ctx.enter_context(tc.tile_pool(name="obuf", bufs=2))
        stats = ctx.enter_context(tc.tile_pool(name="stats", bufs=1))
        small = ctx.enter_context(tc.tile_pool(name="small", bufs=2))
        ps1 = ctx.enter_context(tc.tile_pool(name="ps1", bufs=2, space="PSUM"))
        ps2 = ctx.enter_context(tc.tile_pool(name="ps2", bufs=3, space="PSUM"))
        psg = ctx.enter_context(tc.tile_pool(name="psg", bufs=1, space="PSUM"))
        dram = ctx.enter_context(tc.tile_pool(name="dram", bufs=1, space="DRAM"))
        # ---- constants ----
        # packed conv weights: pairs (dy=-1,dx)+(dy=0,dx) on 128 partitions
        # (idx 0..2 img A, 3..5 img B); singles (dy=+1,dx) on partitions 64:128
        wpk_t = consts.tile([128, 8, 128], BF16)
        nc.sync.dma_start(wpk_t, ins["wpk"])
        wsg_t = consts.tile([128, 2, 128], BF16)
        nc.sync.dma_start(wsg_t, ins["wsg"])
        wva_t = consts.tile([64, 64], F32)
        nc.sync.dma_start(wva_t, ins["wva"])
        wvb_t = consts.tile([64, 64], F32)
        nc.sync.dma_start(wvb_t, ins["wvb"])
        w1t_t = consts.tile([64, 64], F32)
        nc.sync.dma_start(w1t_t, ins["w1t"])
        w2t_t = consts.tile([64, 64], F32)
        nc.sync.dma_start(w2t_t, ins["w2t"])
        cat_t = consts.tile([64, 64], F32)
        nc.sync.dma_start(cat_t, ins["cat"])
        cbt_t = consts.tile([64, 64], F32)
        nc.sync.dma_start(cbt_t, ins["cbt"])
        dwva_t = consts.tile([64, 9], F32)
        nc.sync.dma_start(dwva_t, ins["dwva"])
        dwvb_t = consts.tile([64, 9], F32)
        nc.sync.dma_start(dwvb_t, ins["dwvb"])
        tva_t = consts.tile([64, 1], F32)
        nc.sync.dma_start(tva_t, ins["tva"])
        tvb_t = consts.tile([64, 1], F32)
        nc.sync.dma_start(tvb_t, ins["tvb"])
        hmask_t = consts.tile([64, 64], F32)
        nc.sync.dma_start(hmask_t, ins["hmask"])
        ident = consts.tile([128, 128], F32)
        make_identity(nc, ident)
        ident_bf = consts.tile([128, 128], BF16)
        make_identity(nc, ident_bf)

        # ---- stats accumulators ----
        na = stats.tile([128, rows // 2], F32)
        nb = stats.tile([128, rows // 2], F32)
        junk_a = stats.tile([128, 2, w], BF16)
        junk_b = stats.tile([128, 2, w], BF16)
        # right half accumulates junk (wide-rhs trick to hide gram ldweights)
        gram_ps = psg.tile([128, 256], F32)

        # ================= PASS 1: qk + stats =================
        # xdup/ydup: partitions 0:64 = padded rows r0+i, 64:128 = rows r0+i+1.
        # Packed matmul (128-contraction) covers taps (dy=-1,dx)+(dy=0,dx);
        # singles (dy=+1,dx) read the bottom half one row later.
        xdup = ins["xdup"]
        ydup = ins["ydup"]
        xcol = ins["xcol"]
        ycol = ins["ycol"]
        for b in range(nblk):
            xd = xin.tile([128, blk + 2, wp], BF16, tag="xd")
            nc.sync.dma_start(xd, xdup[:, b * blk : b * blk + blk + 2, :])
            yd = xin.tile([128, blk + 2, wp], BF16, tag="yd")
            nc.sync.dma_start(yd, ydup[:, b * blk : b * blk + blk + 2, :])
            xc = xin.tile([128, blk + 2, wp], BF16, tag="xc")
            nc.sync.dma_start(xc, xcol[:, b * blk : b * blk + blk + 2, :])
            yc = xin.tile([128, blk + 2, wp], BF16, tag="yc")
            nc.sync.dma_start(yc, ycol[:, b * blk : b * blk + blk + 2, :])
            qa_bf = qkev.tile([128, blk, w], BF16)
            qb_bf = qkev.tile([128, blk, w], BF16)
            for g in range(blk // 2):
                j = 2 * g
                grp = (b * blk + j) // 2
                pA = ps1.tile([128, 2, w], F32, tag="pA")
                pB = ps1.tile([128, 2, w], F32, tag="pB")
                for src, cs, pp, w0, w1 in ((xd, xc, pA, 0, 0),
                                            (yd, yc, pB, 4, 1)):
                    for i, dx in enumerate((-1, 0, 1)):
                        nc.tensor.matmul(
                            pp,
                            lhsT=wpk_t[:, w0 + i, :],
                            rhs=src[:, j : j + 2, 1 + dx : 1 + dx + w],
                            start=(i == 0),
                            stop=False,
                        )
                    # pair (dy=+1,dx=-1)+(dy=+1,dx=0) via the col-shifted dup
                    nc.tensor.matmul(
                        pp,
                        lhsT=wpk_t[:, w0 + 3, :],
                        rhs=cs[:, j + 2 : j + 4, 0:w],
                        start=False,
                        stop=False,
                    )
                    # last single (dy=+1,dx=+1) from the top half
                    nc.tensor.matmul(
                        pp,
                        lhsT=wsg_t[0:64, w1, :],
                        rhs=cs[0:64, j + 2 : j + 4, 2 : 2 + w],
                        start=False,
                        stop=True,
                    )
                # norms (sum over pixels of q^2 / k^2) on ScalarE
                nc.scalar.activation(
                    junk_a, pA, mybir.ActivationFunctionType.Square,
                    accum_out=na[:, grp : grp + 1],
                )
                nc.scalar.activation(
                    junk_b, pB, mybir.ActivationFunctionType.Square,
                    accum_out=nb[:, grp : grp + 1],
                )
                # evacuate to bf16 for the Gram
                nc.vector.tensor_copy(qa_bf[:, j : j + 2, :], pA)
                nc.vector.tensor_copy(qb_bf[:, j : j + 2, :], pB)
            # blocked transpose via PE (bf16), evac alternating DVE/ACT
            qaT = qkt.tile([128, nch_blk, 128], BF16)
            qbT = qkt.tile([128, nch_blk, 128], BF16)
            qa_fl = qa_bf.rearrange("p a b -> p (a b)")
            qb_fl = qb_bf.rearrange("p a b -> p (a b)")
            for cc in range(0, nch_blk, 2):
                tpa = ps2.tile([128, 2, 128], BF16, tag="p2")
                nc.tensor.transpose(tpa[:, 0, :],
                                    qa_fl[:, cc * 128 : (cc + 1) * 128],
                                    ident_bf)
                nc.tensor.transpose(tpa[:, 1, :],
                                    qa_fl[:, (cc + 1) * 128 : (cc + 2) * 128],
                                    ident_bf)
                tpb = ps2.tile([128, 2, 128], BF16, tag="p2")
                nc.tensor.transpose(tpb[:, 0, :],
                                    qb_fl[:, cc * 128 : (cc + 1) * 128],
                                    ident_bf)
                nc.tensor.transpose(tpb[:, 1, :],
                                    qb_fl[:, (cc + 1) * 128 : (cc + 2) * 128],
                                    ident_bf)
                if cc % 4 == 0:
                    nc.vector.tensor_copy(qaT[:, cc : cc + 2, :], tpa)
                    nc.scalar.copy(qbT[:, cc : cc + 2, :], tpb)
                else:
                    nc.scalar.copy(qaT[:, cc : cc + 2, :], tpa)
                    nc.vector.tensor_copy(qbT[:, cc : cc + 2, :], tpb)
            for cc in range(nch_blk):
                if cc < nch_blk - 1:
                    nc.tensor.matmul(
                        gram_ps,
                        lhsT=qaT[:, cc, :],
                        rhs=qbT[:, cc : cc + 2, :],
                        start=(b == 0 and cc == 0),
                        stop=False,
                    )
                else:
                    nc.tensor.matmul(
                        gram_ps[:, 0:128],
                        lhsT=qaT[:, cc, :],
                        rhs=qbT[:, cc, :],
                        start=False,
                        stop=(b == nblk - 1),
                    )

        # ---- finalize + allreduce stats ----
        nsum = stats.tile([128, 2], F32)
        nc.vector.tensor_reduce(nsum[:, 0:1], na, axis=mybir.AxisListType.X,
                                op=mybir.AluOpType.add)
        nc.vector.tensor_reduce(nsum[:, 1:2], nb, axis=mybir.AxisListType.X,
                                op=mybir.AluOpType.add)
        stpack = stats.tile([128, 130], F32)
        nc.vector.tensor_copy(stpack[:, 0:128], gram_ps[:, 0:128])
        nc.vector.tensor_copy(stpack[:, 128:130], nsum)
        bounce_in = dram.tile([128, 130], F32)
        bounce_out = dram.tile([128, 130], F32)
        nc.sync.dma_start(bounce_in, stpack)
        nc.gpsimd.collective_compute(
            "AllReduce",
            mybir.AluOpType.add,
            replica_groups=groups,
            ins=[bounce_in.opt()],
            outs=[bounce_out.opt()],
        )
        stall = stats.tile([128, 130], F32)
        nc.sync.dma_start(stall, bounce_out)
        if "dbg" in outs:
            nc.sync.dma_start(outs["dbg"], stall)

        # ---- softmax + fold (tiny) ----
        # stall[:, 0:128] = Gram out[chA, chB]; chA rows = (qa 0:64 | ka 64:128),
        # chB cols = (qb 0:64 | kb 64:128).
        #   S_b  = stall[0:64, 64:128]   (qa . kb)  rows=qa
        #   S_aT = stall[64:128, 0:64]   (ka . qb)  rows=ka
        # col 128 = img-A sumsq (qa|ka), col 129 = img-B sumsq (qb|kb)
        rn = stats.tile([128, 2], F32)
        nc.scalar.activation(rn, stall[:, 128:130],
                             mybir.ActivationFunctionType.Sqrt)
        nc.vector.reciprocal(rn, rn)

        ident64 = ident[0:64, 0:64]

        def softmax_bd(scores_full, name):
            # scores_full: [64,64] sbuf; per-head block-diag softmax -> [64,8]
            masked = stats.tile([64, 64], F32, tag=f"masked_{name}")
            nc.vector.tensor_mul(masked, scores_full, hmask_t)
            sbd = stats.tile([64, 8], F32, tag=f"sbd_{name}")
            nc.vector.tensor_copy(sbd, masked[:, 0:8])
            for h in range(1, HEADS):
                nc.vector.tensor_add(sbd, sbd, masked[:, h * 8 : (h + 1) * 8])
            mx = stats.tile([64, 1], F32, tag=f"mx_{name}")
            se = stats.tile([64, 1], F32, tag=f"se_{name}")
            nc.vector.tensor_reduce(mx, sbd, axis=mybir.AxisListType.X,
                                    op=mybir.AluOpType.max)
            nc.vector.tensor_scalar_sub(sbd, sbd, mx)
            nc.scalar.activation(sbd, sbd, mybir.ActivationFunctionType.Exp,
                                 accum_out=se)
            nc.vector.reciprocal(se, se)
            nc.vector.tensor_scalar_mul(sbd, sbd, se)
            return sbd

        # scores_a: transpose S_aT -> [qb, ka]; scale rows(ka) first, then rows(qb)
        sa_t = stats.tile([64, 64], F32)
        nc.vector.tensor_scalar_mul(sa_t, stall[64:128, 0:64], rn[64:128, 0:1])
        paT = ps2.tile([64, 64], F32, tag="p2")
        nc.tensor.transpose(paT, sa_t, ident64)
        rqa_scale = stats.tile([64, 1], F32)
        nc.vector.tensor_mul(rqa_scale, rn[0:64, 1:2], tva_t)  # rn_qb * temp
        sa_full = stats.tile([64, 64], F32)
        nc.vector.tensor_scalar_mul(sa_full, paT, rqa_scale)
        attn_a = softmax_bd(sa_full, "a")

        # scores_b: S_b rows=qa; col-scale by rn_kb via double transpose
        sbT = ps2.tile([64, 64], F32, tag="p2")
        nc.tensor.transpose(sbT, stall[0:64, 64:128], ident64)
        sb_t = stats.tile([64, 64], F32)
        nc.vector.tensor_scalar_mul(sb_t, sbT, rn[64:128, 1:2])  # rows kb
        sb_ps = ps2.tile([64, 64], F32, tag="p2")
        nc.tensor.transpose(sb_ps, sb_t, ident64)
        rqb_scale = stats.tile([64, 1], F32)
        nc.vector.tensor_mul(rqb_scale, rn[0:64, 0:1], tvb_t)  # rn_qa * (-temp)
        sb_full = stats.tile([64, 64], F32)
        nc.vector.tensor_scalar_mul(sb_full, sb_ps, rqb_scale)
        attn_b = softmax_bd(sb_full, "b")

        # fold: S2 stationaries for pass 2
        s2 = consts.tile([128, 10, 64], BF16)

        def fold_side(attn, w1t_c, wv_c, dwv_c, prow, name):
            bd = stats.tile([64, 64], F32, tag=f"bd_{name}")
            for h in range(HEADS):
                nc.vector.tensor_copy(bd[:, h * 8 : (h + 1) * 8], attn)
            nc.vector.tensor_mul(bd, bd, hmask_t)
            m_ps = ps2.tile([64, 64], F32, tag="p2")
            nc.tensor.matmul(m_ps, lhsT=w1t_c, rhs=bd, start=True, stop=True)
            m_sb = stats.tile([64, 64], F32, tag=f"msb_{name}")
            nc.vector.tensor_copy(m_sb, m_ps)
            mT_ps = ps2.tile([64, 64], F32, tag="p2")
            nc.tensor.transpose(mT_ps, m_sb, ident64)
            mT = stats.tile([64, 64], F32, tag=f"mT_{name}")
            nc.vector.tensor_copy(mT, mT_ps)  # [d, o]
            for t in range(9):
                tmp = small.tile([64, 64], F32, tag=f"tmp_{name}")
                nc.vector.tensor_scalar_mul(tmp, mT, dwv_c[:, t : t + 1])
                s2ps = ps2.tile([64, 64], F32, tag="p2")
                nc.tensor.matmul(s2ps, lhsT=wv_c, rhs=tmp, start=True, stop=True)
                nc.vector.tensor_copy(s2[prow : prow + 64, t, :], s2ps)

        fold_side(attn_a, w1t_t, wva_t, dwva_t, 0, "a")
        fold_side(attn_b, w2t_t, wvb_t, dwvb_t, 64, "b")
        nc.vector.tensor_copy(s2[0:64, 9, :], cat_t)
        nc.vector.tensor_copy(s2[64:128, 9, :], cbt_t)

        # ================= PASS 2: output =================
        evac_engines = [
            lambda o, i: nc.vector.tensor_copy(o, i),
            lambda o, i: nc.scalar.copy(o, i),
        ]
        for b in range(nblk):
            xt2 = xin.tile([128, blk + 2, wp], BF16, tag="xt")
            nc.sync.dma_start(xt2, xy[:, b * blk : b * blk + blk + 2, :])
            ob = obuf.tile([64, blk, w], F32)
            for j in range(blk):
                p2 = ps2.tile([64, w], F32, tag="p2")
                for g in range(10):
                    dy, dx = TAPS[g] if g < 9 else (0, 0)
                    nc.tensor.matmul(
                        p2,
                        lhsT=s2[:, g, :],
                        rhs=xt2[:, j + 1 + dy, 1 + dx : 1 + dx + w],
                        start=(g == 0),
                        stop=(g == 9),
                    )
                evac_engines[j % 2](ob[:, j, :], p2)
            nc.sync.dma_start(out_d[:, b * blk : (b + 1) * blk, :], ob)


# ---------------------------------------------------------------------------
# host side
# ---------------------------------------------------------------------------

def prep_weights(inputs):
    f = lambda k: np.asarray(inputs[k], np.float32)
    qkv_A_w, qkv_B_w = f("qkv_A_w"), f("qkv_B_w")
    dw_A, dw_B = f("dw_A_w")[:, 0], f("dw_B_w")[:, 0]    # [192, 3, 3]
    proj_A, proj_B = f("proj_A_w"), f("proj_B_w")
    concat = f("concat_w")
    temp = f("temperature").reshape(HEADS)

    def fold_qk(qkv_w, dw):
        wqk = qkv_w[:128]            # [128, 64]
        out = np.zeros((64, 9, 128), np.float32)
        for t, (dy, dx) in enumerate(TAPS):
            out[:, t, :] = (wqk * dw[:128, dy + 1, dx + 1][:, None]).T
        return out

    CA, CB = concat[:, :64], concat[:, 64:]
    fa, fb = fold_qk(qkv_A_w, dw_A), fold_qk(qkv_B_w, dw_B)  # [64, 9, 128]
    # taps are ordered (dy,dx) row-major: t = 3*(dy+1) + (dx+1)
    wpk = np.zeros((128, 8, 128), np.float32)
    wsg = np.zeros((128, 2, 128), np.float32)
    for i in range(3):               # dx = i - 1
        wpk[0:64, i] = fa[:, 0 + i]      # (dy=-1, dx)
        wpk[64:128, i] = fa[:, 3 + i]    # (dy= 0, dx)
        wpk[0:64, 4 + i] = fb[:, 0 + i]
        wpk[64:128, 4 + i] = fb[:, 3 + i]
    wpk[0:64, 3] = fa[:, 6]              # (dy=+1, dx=-1)
    wpk[64:128, 3] = fa[:, 7]            # (dy=+1, dx= 0)
    wpk[0:64, 7] = fb[:, 6]
    wpk[64:128, 7] = fb[:, 7]
    wsg[0:64, 0] = fa[:, 8]              # (dy=+1, dx=+1)
    wsg[0:64, 1] = fb[:, 8]
    consts = {
        "wpk": wpk.astype(BF16_NP),
        "wsg": wsg.astype(BF16_NP),
        "wva": np.ascontiguousarray(qkv_A_w[128:192]),   # [d, xc]
        "wvb": np.ascontiguousarray(qkv_B_w[128:192]),
        "w1t": np.ascontiguousarray((CA @ proj_A).T),
        "w2t": np.ascontiguousarray((CB @ proj_B).T),
        "cat": np.ascontiguousarray(CA.T),
        "cbt": np.ascontiguousarray(CB.T),
        "dwva": np.ascontiguousarray(dw_A[128:192].reshape(64, 9)),
        "dwvb": np.ascontiguousarray(dw_B[128:192].reshape(64, 9)),
        "tva": np.repeat(temp, CH).reshape(64, 1).astype(np.float32),
        "tvb": (-np.repeat(temp, CH)).reshape(64, 1).astype(np.float32),
        "hmask": np.kron(np.eye(HEADS, dtype=np.float32),
                         np.ones((CH, CH), np.float32)),
    }
    return consts


def shard_inputs(inputs):
    x = np.asarray(inputs["x"], np.float32)
    y = np.asarray(inputs["y"], np.float32)
    b, c, h, w = x.shape
    xp = np.zeros((b, c, h + 2, w + 2), np.float32)
    yp = np.zeros((b, c, h + 2, w + 2), np.float32)
    xp[:, :, 1 : h + 1, 1 : w + 1] = x
    yp[:, :, 1 : h + 1, 1 : w + 1] = y
    consts = prep_weights(inputs)
    in_maps = []
    rloc = h // 2
    for core in range(N_CORES):
        bi, half = core // 2, core % 2
        r0 = half * rloc
        xy = np.concatenate(
            [xp[bi, :, r0 : r0 + rloc + 2, :], yp[bi, :, r0 : r0 + rloc + 2, :]],
            axis=0,
        )
        # dup layouts: partitions 0:64 = padded rows r0+i, 64:128 = r0+i+1
        xdup = np.zeros((128, rloc + 2, w + 2), np.float32)
        ydup = np.zeros((128, rloc + 2, w + 2), np.float32)
        hi = min(r0 + rloc + 3, h + 2)
        xdup[0:64] = xp[bi, :, r0 : r0 + rloc + 2, :]
        xdup[64:128, : hi - r0 - 1] = xp[bi, :, r0 + 1 : hi, :]
        ydup[0:64] = yp[bi, :, r0 : r0 + rloc + 2, :]
        ydup[64:128, : hi - r0 - 1] = yp[bi, :, r0 + 1 : hi, :]
        # col-shifted dups: top = padded rows r0+i, bottom = same rows col+1
        xcol = np.zeros((128, rloc + 2, w + 2), np.float32)
        ycol = np.zeros((128, rloc + 2, w + 2), np.float32)
        xcol[0:64] = xp[bi, :, r0 : r0 + rloc + 2, :]
        xcol[64:128, :, : w + 1] = xp[bi, :, r0 : r0 + rloc + 2, 1:]
        ycol[0:64] = yp[bi, :, r0 : r0 + rloc + 2, :]
        ycol[64:128, :, : w + 1] = yp[bi, :, r0 : r0 + rloc + 2, 1:]
        m = {"xy": np.ascontiguousarray(xy).astype(BF16_NP),
             "xdup": xdup.astype(BF16_NP), "ydup": ydup.astype(BF16_NP),
             "xcol": xcol.astype(BF16_NP), "ycol": ycol.astype(BF16_NP)}
        m.update(consts)
        in_maps.append(m)
    return in_maps


_CACHE = {}


def build_program(cfg):
    key = tuple(sorted(cfg.items())) if not isinstance(cfg, tuple) else cfg
    key = (cfg["rows"], cfg["blk"], cfg["w"], len(cfg["groups"]))
    if key in _CACHE:
        return _CACHE[key]
    nc = bacc.Bacc("TRN2", target_bir_lowering=False, debug=False,
                   num_devices=cfg["n_cores"])
    rows, w = cfg["rows"], cfg["w"]
    ins = {
        "xy": nc.dram_tensor("xy", [128, rows + 2, w + 2], BF16,
                             kind="ExternalInput").ap(),
        "xdup": nc.dram_tensor("xdup", [128, rows + 2, w + 2], BF16,
                               kind="ExternalInput").ap(),
        "ydup": nc.dram_tensor("ydup", [128, rows + 2, w + 2], BF16,
                               kind="ExternalInput").ap(),
        "xcol": nc.dram_tensor("xcol", [128, rows + 2, w + 2], BF16,
                               kind="ExternalInput").ap(),
        "ycol": nc.dram_tensor("ycol", [128, rows + 2, w + 2], BF16,
                               kind="ExternalInput").ap(),
        "wpk": nc.dram_tensor("wpk", [128, 8, 128], BF16,
                              kind="ExternalInput").ap(),
        "wsg": nc.dram_tensor("wsg", [128, 2, 128], BF16,
                              kind="ExternalInput").ap(),
        "wva": nc.dram_tensor("wva", [64, 64], F32, kind="ExternalInput").ap(),
        "wvb": nc.dram_tensor("wvb", [64, 64], F32, kind="ExternalInput").ap(),
        "w1t": nc.dram_tensor("w1t", [64, 64], F32, kind="ExternalInput").ap(),
        "w2t": nc.dram_tensor("w2t", [64, 64], F32, kind="ExternalInput").ap(),
        "cat": nc.dram_tensor("cat", [64, 64], F32, kind="ExternalInput").ap(),
        "cbt": nc.dram_tensor("cbt", [64, 64], F32, kind="ExternalInput").ap(),
        "dwva": nc.dram_tensor("dwva", [64, 9], F32, kind="ExternalInput").ap(),
        "dwvb": nc.dram_tensor("dwvb", [64, 9], F32, kind="ExternalInput").ap(),
        "tva": nc.dram_tensor("tva", [64, 1], F32, kind="ExternalInput").ap(),
        "tvb": nc.dram_tensor("tvb", [64, 1], F32, kind="ExternalInput").ap(),
        "hmask": nc.dram_tensor("hmask", [64, 64], F32,
                                kind="ExternalInput").ap(),
    }
    outs = {
        "out": nc.dram_tensor("out", [64, rows, w], F32,
                              kind="ExternalOutput").ap(),
    }
    with tile.TileContext(nc) as tc:
        kernel_body(tc, outs, ins, cfg)
    nc.compile()
    _CACHE[key] = nc
    return nc


def default_cfg():
    return {
        "rows": R_LOC,
        "blk": BLK,
        "w": W,
        "n_cores": N_CORES,
        "groups": GROUPS,
    }


def _run(inputs, trace=False):
    cfg = default_cfg()
    nc = build_program(cfg)
    in_maps = shard_inputs(inputs)
    res = run_bass_kernel_spmd(nc, in_maps, core_ids=list(range(N_CORES)),
                               trace=trace)
    x = np.asarray(inputs["x"])
    b, c, h, w = x.shape
    out = np.empty((b, c, h, w), np.float32)
    rloc = h // 2
    for core in range(N_CORES):
        bi, half = core // 2, core % 2
        out[bi, :, half * rloc : (half + 1) * rloc, :] = res.results[core]["out"]
    return out, res


def kernel(**inputs):
    out, _ = _run(inputs, trace=False)
    return out



# revision 31
# speedup vs baseline: 1.1432x; 1.1432x over previous
"""Trainium2 Bass kernel for nn_Cross_Attention (dual cross channel-attention block).

Architecture (8 NeuronCores, data-parallel):
  core i -> (batch b = i//2, row-half h = i%2) of the 4x[64,256,256] images.

Math restructuring (exact, up to float assoc; bf16 streaming operands):
  qkv = dwconv3x3(conv1x1(x, W))  is computed with the 3x3 depthwise conv
  *folded* into the 1x1 conv, and the 9 taps are packed into 5 PSUM-accumulated
  128-contraction matmuls per 2-row group using host-prepped duplicate layouts
  (xdup/ydup: bottom 64 partitions hold the image shifted +1 row; xcol/ycol:
  shifted +1 col), each streaming a 512-wide (2 rows x 256 cols) window.
  Channel attention needs only second moments of q,k:
     S_a[c,d] = sum_p qb[c,p] ka[d,p],  S_b[c,d] = sum_p qa[c,p] kb[d,p]
     n_*[c]   = sum_p q[c,p]^2
  computed on-chip (Gram via DMA-transposed bf16 operands + PE matmuls,
  norms via ScalarE Square+accum), then AllReduce'd across the 2 cores
  sharing a batch. Softmax + all downstream linear layers are folded into
  10 per-batch [128,64] stationaries applied in one output pass:
     out = sum_t S2A_t^T @ x_shift_t + S2B_t^T @ y_shift_t + CA^T@x + CB^T@y
  where S2A_t[xc,o] = sum_d WvA[d,xc] * (W1 @ blockdiag(attn_a))[o,d] * dwvA[d,t]
  and W1 = concat_w[:, :64] @ proj_A_w  (host-precomputed), etc.
"""

import os
import sys

sys.path.insert(0, "/opt/trn_rl_repo")

import ml_dtypes
import numpy as np

BF16_NP = ml_dtypes.bfloat16

import concourse.bass as bass
import concourse.bacc as bacc
import concourse.tile as tile
from concourse import mybir
from concourse.bass_utils import run_bass_kernel_spmd
from concourse.masks import make_identity

F32 = mybir.dt.float32
F32R = mybir.dt.float32r
BF16 = mybir.dt.bfloat16

B, C, H, W = 4, 64, 256, 256
HEADS, CH = 8, 8
WP = W + 2          # zero-padded width
N_CORES = 8
R_LOC = H // 2      # output rows per core
BLK = 8             # rows per streaming block
TAPS = [(dy, dx) for dy in (-1, 0, 1) for dx in (-1, 0, 1)]
GROUPS = [[0, 1], [2, 3], [4, 5], [6, 7]]


def kernel_body(tc, outs, ins, cfg):
    nc = tc.nc
    rows = cfg["rows"]
    blk = cfg["blk"]
    nblk = rows // blk
    w = cfg["w"]
    wp = w + 2
    groups = cfg["groups"]
    nch_blk = blk * w // 128  # 128-px transpose chunks per block

    xy = ins["xy"]            # [128, rows+2, wp] dram (x on 0:64, y on 64:128)
    out_d = outs["out"]       # [64, rows, w] dram

    from contextlib import ExitStack

    with ExitStack() as ctx:
        consts = ctx.enter_context(tc.tile_pool(name="consts", bufs=1))
        xin = ctx.enter_context(tc.tile_pool(name="xin", bufs=2))
        qkev = ctx.enter_context(tc.tile_pool(name="qkev", bufs=2))
        qkt = ctx.enter_context(tc.tile_pool(name="qkt", bufs=2))
        obuf = ctx.enter_context(tc.tile_pool(name="obuf", bufs=2))
        stats = ctx.enter_context(tc.tile_pool(name="stats", bufs=1))
        small = ctx.enter_context(tc.tile_pool(name="small", bufs=2))
        ps1 = ctx.enter_context(tc.tile_pool(name="ps1", bufs=2, space="PSUM"))
        ps2 = ctx.enter_context(tc.tile_pool(name="ps2", bufs=3, space="PSUM"))
        psg = ctx.enter_context(tc.tile_pool(name="psg", bufs=1, space="PSUM"))
        dram = ctx.enter_context(tc.tile_pool(name="dram", bufs=1, space="DRAM"))
        # ---- constants ----
        # packed conv weights: pairs (dy=-1,dx)+(dy=0,dx) on 128 partitions
        # (idx 0..2 img A, 3..5 img B); singles (dy=+1,dx) on partitions 64:128
        wpk_t = consts.tile([128, 8, 128], BF16)
        nc.sync.dma_start(wpk_t, ins["wpk"])
        wsg_t = consts.tile([128, 2, 128], BF16)
        nc.sync.dma_start(wsg_t, ins["wsg"])
        wva_t = consts.tile([64, 64], F32)
        nc.sync.dma_start(wva_t, ins["wva"])
        wvb_t = consts.tile([64, 64], F32)
        nc.sync.dma_start(wvb_t, ins["wvb"])
        w1t_t = consts.tile([64, 64], F32)
        nc.sync.dma_start(w1t_t, ins["w1t"])
        w2t_t = consts.tile([64, 64], F32)
        nc.sync.dma_start(w2t_t, ins["w2t"])
        cat_t = consts.tile([64, 64], F32)
        nc.sync.dma_start(cat_t, ins["cat"])
        cbt_t = consts.tile([64, 64], F32)
        nc.sync.dma_start(cbt_t, ins["cbt"])
        dwva_t = consts.tile([64, 9], F32)
        nc.sync.dma_start(dwva_t, ins["dwva"])
        dwvb_t = consts.tile([64, 9], F32)
        nc.sync.dma_start(dwvb_t, ins["dwvb"])
        tva_t = consts.tile([64, 1], F32)
        nc.sync.dma_start(tva_t, ins["tva"])
        tvb_t = consts.tile([64, 1], F32)
        nc.sync.dma_start(tvb_t, ins["tvb"])
        hmask_t = consts.tile([64, 64], F32)
        nc.sync.dma_start(hmask_t, ins["hmask"])
        ident = consts.tile([128, 128], F32)
        make_identity(nc, ident)
        ident_bf = consts.tile([128, 128], BF16)
        make_identity(nc, ident_bf)

        # ---- stats accumulators ----
        na = stats.tile([128, rows // 2], F32)
        nb = stats.tile([128, rows // 2], F32)
        junk_a = stats.tile([128, 2, w], BF16)
        junk_b = stats.tile([128, 2, w], BF16)
        # right half accumulates junk (wide-rhs trick to hide gram ldweights)
        gram_ps = psg.tile([128, 256], F32)

        # ================= PASS 1: qk + stats =================
        # xdup/ydup: partitions 0:64 = padded rows r0+i, 64:128 = rows r0+i+1.
        # Packed matmul (128-contraction) covers taps (dy=-1,dx)+(dy=0,dx);
        # singles (dy=+1,dx) read the bottom half one row later.
        xdup = ins["xdup"]
        ydup = ins["ydup"]
        xcol = ins["xcol"]
        ycol = ins["ycol"]
        for b in range(nblk):
            xd = xin.tile([128, blk + 2, wp], BF16, tag="xd")
            nc.sync.dma_start(xd, xdup[:, b * blk : b * blk + blk + 2, :])
            yd = xin.tile([128, blk + 2, wp], BF16, tag="yd")
            nc.sync.dma_start(yd, ydup[:, b * blk : b * blk + blk + 2, :])
            xc = xin.tile([128, blk + 2, wp], BF16, tag="xc")
            nc.sync.dma_start(xc, xcol[:, b * blk : b * blk + blk + 2, :])
            yc = xin.tile([128, blk + 2, wp], BF16, tag="yc")
            nc.sync.dma_start(yc, ycol[:, b * blk : b * blk + blk + 2, :])
            qa_bf = qkev.tile([128, blk, w], BF16)
            qb_bf = qkev.tile([128, blk, w], BF16)
            for g in range(blk // 2):
                j = 2 * g
                grp = (b * blk + j) // 2
                pA = ps1.tile([128, 2, w], F32, tag="pA")
                pB = ps1.tile([128, 2, w], F32, tag="pB")
                for src, cs, pp, w0, w1 in ((xd, xc, pA, 0, 0),
                                            (yd, yc, pB, 4, 1)):
                    for i, dx in enumerate((-1, 0, 1)):
                        nc.tensor.matmul(
                            pp,
                            lhsT=wpk_t[:, w0 + i, :],
                            rhs=src[:, j : j + 2, 1 + dx : 1 + dx + w],
                            start=(i == 0),
                            stop=False,
                        )
                    # pair (dy=+1,dx=-1)+(dy=+1,dx=0) via the col-shifted dup
                    nc.tensor.matmul(
                        pp,
                        lhsT=wpk_t[:, w0 + 3, :],
                        rhs=cs[:, j + 2 : j + 4, 0:w],
                        start=False,
                        stop=False,
                    )
                    # last single (dy=+1,dx=+1) from the top half
                    nc.tensor.matmul(
                        pp,
                        lhsT=wsg_t[0:64, w1, :],
                        rhs=cs[0:64, j + 2 : j + 4, 2 : 2 + w],
                        start=False,
                        stop=True,
                    )
                # norms (sum over pixels of q^2 / k^2) on ScalarE
                nc.scalar.activation(
                    junk_a, pA, mybir.ActivationFunctionType.Square,
                    accum_out=na[:, grp : grp + 1],
                )
                nc.scalar.activation(
                    junk_b, pB, mybir.ActivationFunctionType.Square,
                    accum_out=nb[:, grp : grp + 1],
                )
                # evacuate to bf16 for the Gram
                nc.vector.tensor_copy(qa_bf[:, j : j + 2, :], pA)
                nc.vector.tensor_copy(qb_bf[:, j : j + 2, :], pB)
            # blocked transpose via PE (bf16), evac alternating DVE/ACT
            qaT = qkt.tile([128, nch_blk, 128], BF16)
            qbT = qkt.tile([128, nch_blk, 128], BF16)
            qa_fl = qa_bf.rearrange("p a b -> p (a b)")
            qb_fl = qb_bf.rearrange("p a b -> p (a b)")
            for cc in range(0, nch_blk, 2):
                tpa = ps2.tile([128, 2, 128], BF16, tag="p2")
                nc.tensor.transpose(tpa[:, 0, :],
                                    qa_fl[:, cc * 128 : (cc + 1) * 128],
                                    ident_bf)
                nc.tensor.transpose(tpa[:, 1, :],
                                    qa_fl[:, (cc + 1) * 128 : (cc + 2) * 128],
                                    ident_bf)
                tpb = ps2.tile([128, 2, 128], BF16, tag="p2")
                nc.tensor.transpose(tpb[:, 0, :],
                                    qb_fl[:, cc * 128 : (cc + 1) * 128],
                                    ident_bf)
                nc.tensor.transpose(tpb[:, 1, :],
                                    qb_fl[:, (cc + 1) * 128 : (cc + 2) * 128],
                                    ident_bf)
                if cc % 4 == 0:
                    nc.vector.tensor_copy(qaT[:, cc : cc + 2, :], tpa)
                    nc.scalar.copy(qbT[:, cc : cc + 2, :], tpb)
                else:
                    nc.scalar.copy(qaT[:, cc : cc + 2, :], tpa)
                    nc.vector.tensor_copy(qbT[:, cc : cc + 2, :], tpb)
            for cc in range(nch_blk):
                if cc < nch_blk - 1:
                    nc.tensor.matmul(
                        gram_ps,
                        lhsT=qaT[:, cc, :],
                        rhs=qbT[:, cc : cc + 2, :],
                        start=(b == 0 and cc == 0),
                        stop=False,
                    )
                else:
                    nc.tensor.matmul(
                        gram_ps[:, 0:128],
                        lhsT=qaT[:, cc, :],
                        rhs=qbT[:, cc, :],
                        start=False,
                        stop=(b == nblk - 1),
                    )

        # ---- finalize + allreduce stats ----
        nsum = stats.tile([128, 2], F32)
        nc.vector.tensor_reduce(nsum[:, 0:1], na, axis=mybir.AxisListType.X,
                                op=mybir.AluOpType.add)
        nc.vector.tensor_reduce(nsum[:, 1:2], nb, axis=mybir.AxisListType.X,
                                op=mybir.AluOpType.add)
        stpack = stats.tile([128, 130], F32)
        nc.vector.tensor_copy(stpack[:, 0:128], gram_ps[:, 0:128])
        nc.vector.tensor_copy(stpack[:, 128:130], nsum)
        bounce_in = dram.tile([128, 130], F32)
        bounce_out = dram.tile([128, 130], F32)
        nc.sync.dma_start(bounce_in, stpack)
        nc.gpsimd.collective_compute(
            "AllReduce",
            mybir.AluOpType.add,
            replica_groups=groups,
            ins=[bounce_in.opt()],
            outs=[bounce_out.opt()],
        )
        stall = stats.tile([128, 130], F32)
        nc.sync.dma_start(stall, bounce_out)
        if "dbg" in outs:
            nc.sync.dma_start(outs["dbg"], stall)

        # ---- softmax + fold (tiny) ----
        # stall[:, 0:128] = Gram out[chA, chB]; chA rows = (qa 0:64 | ka 64:128),
        # chB cols = (qb 0:64 | kb 64:128).
        #   S_b  = stall[0:64, 64:128]   (qa . kb)  rows=qa
        #   S_aT = stall[64:128, 0:64]   (ka . qb)  rows=ka
        # col 128 = img-A sumsq (qa|ka), col 129 = img-B sumsq (qb|kb)
        rn = stats.tile([128, 2], F32)
        nc.scalar.activation(rn, stall[:, 128:130],
                             mybir.ActivationFunctionType.Sqrt)
        nc.vector.reciprocal(rn, rn)

        ident64 = ident[0:64, 0:64]

        def softmax_bd(scores_full, name):
            # scores_full: [64,64] sbuf; per-head block-diag softmax -> [64,8]
            masked = stats.tile([64, 64], F32, tag=f"masked_{name}")
            nc.vector.tensor_mul(masked, scores_full, hmask_t)
            sbd = stats.tile([64, 8], F32, tag=f"sbd_{name}")
            nc.vector.tensor_copy(sbd, masked[:, 0:8])
            for h in range(1, HEADS):
                nc.vector.tensor_add(sbd, sbd, masked[:, h * 8 : (h + 1) * 8])
            mx = stats.tile([64, 1], F32, tag=f"mx_{name}")
            se = stats.tile([64, 1], F32, tag=f"se_{name}")
            nc.vector.tensor_reduce(mx, sbd, axis=mybir.AxisListType.X,
                                    op=mybir.AluOpType.max)
            nc.vector.tensor_scalar_sub(sbd, sbd, mx)
            nc.scalar.activation(sbd, sbd, mybir.ActivationFunctionType.Exp,
                                 accum_out=se)
            nc.vector.reciprocal(se, se)
            nc.vector.tensor_scalar_mul(sbd, sbd, se)
            return sbd

        # scores_a: transpose S_aT -> [qb, ka]; scale rows(ka) first, then rows(qb)
        sa_t = stats.tile([64, 64], F32)
        nc.vector.tensor_scalar_mul(sa_t, stall[64:128, 0:64], rn[64:128, 0:1])
        paT = ps2.tile([64, 64], F32, tag="p2")
        nc.tensor.transpose(paT, sa_t, ident64)
        rqa_scale = stats.tile([64, 1], F32)
        nc.vector.tensor_mul(rqa_scale, rn[0:64, 1:2], tva_t)  # rn_qb * temp
        sa_full = stats.tile([64, 64], F32)
        nc.vector.tensor_scalar_mul(sa_full, paT, rqa_scale)
        attn_a = softmax_bd(sa_full, "a")

        # scores_b: S_b rows=qa; col-scale by rn_kb via double transpose
        sbT = ps2.tile([64, 64], F32, tag="p2")
        nc.tensor.transpose(sbT, stall[0:64, 64:128], ident64)
        sb_t = stats.tile([64, 64], F32)
        nc.vector.tensor_scalar_mul(sb_t, sbT, rn[64:128, 1:2])  # rows kb
        sb_ps = ps2.tile([64, 64], F32, tag="p2")
        nc.tensor.transpose(sb_ps, sb_t, ident64)
        rqb_scale = stats.tile([64, 1], F32)
        nc.vector.tensor_mul(rqb_scale, rn[0:64, 0:1], tvb_t)  # rn_qa * (-temp)
        sb_full = stats.tile([64, 64], F32)
        nc.vector.tensor_scalar_mul(sb_full, sb_ps, rqb_scale)
        attn_b = softmax_bd(sb_full, "b")

        # fold: S2 stationaries for pass 2
        s2 = consts.tile([128, 10, 64], BF16)

        def fold_side(attn, w1t_c, wv_c, dwv_c, prow, name):
            bd = stats.tile([64, 64], F32, tag=f"bd_{name}")
            for h in range(HEADS):
                nc.vector.tensor_copy(bd[:, h * 8 : (h + 1) * 8], attn)
            nc.vector.tensor_mul(bd, bd, hmask_t)
            m_ps = ps2.tile([64, 64], F32, tag="p2")
            nc.tensor.matmul(m_ps, lhsT=w1t_c, rhs=bd, start=True, stop=True)
            m_sb = stats.tile([64, 64], F32, tag=f"msb_{name}")
            nc.vector.tensor_copy(m_sb, m_ps)
            mT_ps = ps2.tile([64, 64], F32, tag="p2")
            nc.tensor.transpose(mT_ps, m_sb, ident64)
            mT = stats.tile([64, 64], F32, tag=f"mT_{name}")
            nc.vector.tensor_copy(mT, mT_ps)  # [d, o]
            for t in range(9):
                tmp = small.tile([64, 64], F32, tag=f"tmp_{name}")
                nc.vector.tensor_scalar_mul(tmp, mT, dwv_c[:, t : t + 1])
                s2ps = ps2.tile([64, 64], F32, tag="p2")
                nc.tensor.matmul(s2ps, lhsT=wv_c, rhs=tmp, start=True, stop=True)
                nc.vector.tensor_copy(s2[prow : prow + 64, t, :], s2ps)

        fold_side(attn_a, w1t_t, wva_t, dwva_t, 0, "a")
        fold_side(attn_b, w2t_t, wvb_t, dwvb_t, 64, "b")
        nc.vector.tensor_copy(s2[0:64, 9, :], cat_t)
        nc.vector.tensor_copy(s2[64:128, 9, :], cbt_t)

        # ================= PASS 2: output =================
        evac_engines = [
            lambda o, i: nc.vector.tensor_copy(o, i),
            lambda o, i: nc.scalar.copy(o, i),
        ]
        for b in range(nblk):
            xt2 = xin.tile([128, blk + 2, wp], BF16, tag="xt")
            nc.sync.dma_start(xt2, xy[:, b * blk : b * blk + blk + 2, :])
            ob = obuf.tile([64, blk, w], F32)
            for j in range(blk):
                p2 = ps2.tile([64, w], F32, tag="p2")
                for g in range(10):
                    dy, dx = TAPS[g] if g < 9 else (0, 0)
                    nc.tensor.matmul(
                        p2,
                        lhsT=s2[:, g, :],
                        rhs=xt2[:, j + 1 + dy, 1 + dx : 1 + dx + w],
                        start=(g == 0),
                        stop=(g == 9),
                    )
                evac_engines[j % 2](ob[:, j, :], p2)
            nc.sync.dma_start(out_d[:, b * blk : (b + 1) * blk, :], ob)


# ---------------------------------------------------------------------------
# host side
# ---------------------------------------------------------------------------

def prep_weights(inputs):
    f = lambda k: np.asarray(inputs[k], np.float32)
    qkv_A_w, qkv_B_w = f("qkv_A_w"), f("qkv_B_w")
    dw_A, dw_B = f("dw_A_w")[:, 0], f("dw_B_w")[:, 0]    # [192, 3, 3]
    proj_A, proj_B = f("proj_A_w"), f("proj_B_w")
    concat = f("concat_w")
    temp = f("temperature").reshape(HEADS)

    def fold_qk(qkv_w, dw):
        wqk = qkv_w[:128]            # [128, 64]
        out = np.zeros((64, 9, 128), np.float32)
        for t, (dy, dx) in enumerate(TAPS):
            out[:, t, :] = (wqk * dw[:128, dy + 1, dx + 1][:, None]).T
        return out

    CA, CB = concat[:, :64], concat[:, 64:]
    fa, fb = fold_qk(qkv_A_w, dw_A), fold_qk(qkv_B_w, dw_B)  # [64, 9, 128]
    # taps are ordered (dy,dx) row-major: t = 3*(dy+1) + (dx+1)
    wpk = np.zeros((128, 8, 128), np.float32)
    wsg = np.zeros((128, 2, 128), np.float32)
    for i in range(3):               # dx = i - 1
        wpk[0:64, i] = fa[:, 0 + i]      # (dy=-1, dx)
        wpk[64:128, i] = fa[:, 3 + i]    # (dy= 0, dx)
        wpk[0:64, 4 + i] = fb[:, 0 + i]
        wpk[64:128, 4 + i] = fb[:, 3 + i]
    wpk[0:64, 3] = fa[:, 6]              # (dy=+1, dx=-1)
    wpk[64:128, 3] = fa[:, 7]            # (dy=+1, dx= 0)
    wpk[0:64, 7] = fb[:, 6]
    wpk[64:128, 7] = fb[:, 7]
    wsg[0:64, 0] = fa[:, 8]              # (dy=+1, dx=+1)
    wsg[0:64, 1] = fb[:, 8]
    consts = {
        "wpk": wpk.astype(BF16_NP),
        "wsg": wsg.astype(BF16_NP),
        "wva": np.ascontiguousarray(qkv_A_w[128:192]),   # [d, xc]
        "wvb": np.ascontiguousarray(qkv_B_w[128:192]),
        "w1t": np.ascontiguousarray((CA @ proj_A).T),
        "w2t": np.ascontiguousarray((CB @ proj_B).T),
        "cat": np.ascontiguousarray(CA.T),
        "cbt": np.ascontiguousarray(CB.T),
        "dwva": np.ascontiguousarray(dw_A[128:192].reshape(64, 9)),
        "dwvb": np.ascontiguousarray(dw_B[128:192].reshape(64, 9)),
        "tva": np.repeat(temp, CH).reshape(64, 1).astype(np.float32),
        "tvb": (-np.repeat(temp, CH)).reshape(64, 1).astype(np.float32),
        "hmask": np.kron(np.eye(HEADS, dtype=np.float32),
                         np.ones((CH, CH), np.float32)),
    }
    return consts


def shard_inputs(inputs):
    x = np.asarray(inputs["x"], np.float32)
    y = np.asarray(inputs["y"], np.float32)
    b, c, h, w = x.shape
    xp = np.zeros((b, c, h + 2, w + 2), np.float32)
    yp = np.zeros((b, c, h + 2, w + 2), np.float32)
    xp[:, :, 1 : h + 1, 1 : w + 1] = x
    yp[:, :, 1 : h + 1, 1 : w + 1] = y
    consts = prep_weights(inputs)
    in_maps = []
    rloc = h // 2
    for core in range(N_CORES):
        bi, half = core // 2, core % 2
        r0 = half * rloc
        xy = np.concatenate(
            [xp[bi, :, r0 : r0 + rloc + 2, :], yp[bi, :, r0 : r0 + rloc + 2, :]],
            axis=0,
        )
        # dup layouts: partitions 0:64 = padded rows r0+i, 64:128 = r0+i+1
        xdup = np.zeros((128, rloc + 2, w + 2), np.float32)
        ydup = np.zeros((128, rloc + 2, w + 2), np.float32)
        hi = min(r0 + rloc + 3, h + 2)
        xdup[0:64] = xp[bi, :, r0 : r0 + rloc + 2, :]
        xdup[64:128, : hi - r0 - 1] = xp[bi, :, r0 + 1 : hi, :]
        ydup[0:64] = yp[bi, :, r0 : r0 + rloc + 2, :]
        ydup[64:128, : hi - r0 - 1] = yp[bi, :, r0 + 1 : hi, :]
        # col-shifted dups: top = padded rows r0+i, bottom = same rows col+1
        xcol = np.zeros((128, rloc + 2, w + 2), np.float32)
        ycol = np.zeros((128, rloc + 2, w + 2), np.float32)
        xcol[0:64] = xp[bi, :, r0 : r0 + rloc + 2, :]
        xcol[64:128, :, : w + 1] = xp[bi, :, r0 : r0 + rloc + 2, 1:]
        ycol[0:64] = yp[bi, :, r0 : r0 + rloc + 2, :]
        ycol[64:128, :, : w + 1] = yp[bi, :, r0 : r0 + rloc + 2, 1:]
        m = {"xy": np.ascontiguousarray(xy).astype(BF16_NP),
             "xdup": xdup.astype(BF16_NP), "ydup": ydup.astype(BF16_NP),
             "xcol": xcol.astype(BF16_NP), "ycol": ycol.astype(BF16_NP)}
        m.update(consts)
        in_maps.append(m)
    return in_maps


_CACHE = {}


def build_program(cfg):
    key = tuple(sorted(cfg.items())) if not isinstance(cfg, tuple) else cfg
    key = (cfg["rows"], cfg["blk"], cfg["w"], len(cfg["groups"]))
    if key in _CACHE:
        return _CACHE[key]
    nc = bacc.Bacc("TRN2", target_bir_lowering=False, debug=False,
                   num_devices=cfg["n_cores"])
    rows, w = cfg["rows"], cfg["w"]
    ins = {
        "xy": nc.dram_tensor("xy", [128, rows + 2, w + 2], BF16,
                             kind="ExternalInput").ap(),
        "xdup": nc.dram_tensor("xdup", [128, rows + 2, w + 2], BF16,
                               kind="ExternalInput").ap(),
        "ydup": nc.dram_tensor("ydup", [128, rows + 2, w + 2], BF16,
                               kind="ExternalInput").ap(),
        "xcol": nc.dram_tensor("xcol", [128, rows + 2, w + 2], BF16,
                               kind="ExternalInput").ap(),
        "ycol": nc.dram_tensor("ycol", [128, rows + 2, w + 2], BF16,
                               kind="ExternalInput").ap(),
        "wpk": nc.dram_tensor("wpk", [128, 8, 128], BF16,
                              kind="ExternalInput").ap(),
        "wsg": nc.dram_tensor("wsg", [128, 2, 128], BF16,
                              kind="ExternalInput").ap(),
        "wva": nc.dram_tensor("wva", [64, 64], F32, kind="ExternalInput").ap(),
        "wvb": nc.dram_tensor("wvb", [64, 64], F32, kind="ExternalInput").ap(),
        "w1t": nc.dram_tensor("w1t", [64, 64], F32, kind="ExternalInput").ap(),
        "w2t": nc.dram_tensor("w2t", [64, 64], F32, kind="ExternalInput").ap(),
        "cat": nc.dram_tensor("cat", [64, 64], F32, kind="ExternalInput").ap(),
        "cbt": nc.dram_tensor("cbt", [64, 64], F32, kind="ExternalInput").ap(),
        "dwva": nc.dram_tensor("dwva", [64, 9], F32, kind="ExternalInput").ap(),
        "dwvb": nc.dram_tensor("dwvb", [64, 9], F32, kind="ExternalInput").ap(),
        "tva": nc.dram_tensor("tva", [64, 1], F32, kind="ExternalInput").ap(),
        "tvb": nc.dram_tensor("tvb", [64, 1], F32, kind="ExternalInput").ap(),
        "hmask": nc.dram_tensor("hmask", [64, 64], F32,
                                kind="ExternalInput").ap(),
    }
    outs = {
        "out": nc.dram_tensor("out", [64, rows, w], F32,
                              kind="ExternalOutput").ap(),
    }
    with tile.TileContext(nc) as tc:
        kernel_body(tc, outs, ins, cfg)
    nc.compile()
    _CACHE[key] = nc
    return nc


def default_cfg():
    return {
        "rows": R_LOC,
        "blk": BLK,
        "w": W,
        "n_cores": N_CORES,
        "groups": GROUPS,
    }


def _run(inputs, trace=False):
    cfg = default_cfg()
    nc = build_program(cfg)
    in_maps = shard_inputs(inputs)
    res = run_bass_kernel_spmd(nc, in_maps, core_ids=list(range(N_CORES)),
                               trace=trace)
    x = np.asarray(inputs["x"])
    b, c, h, w = x.shape
    out = np.empty((b, c, h, w), np.float32)
    rloc = h // 2
    for core in range(N_CORES):
        bi, half = core // 2, core % 2
        out[bi, :, half * rloc : (half + 1) * rloc, :] = res.results[core]["out"]
    return out, res


def kernel(**inputs):
    out, _ = _run(inputs, trace=False)
    return out

